# revision 58
# baseline (speedup 1.0000x reference)
"""Trainium2 8-core kernel for nn_AttnAgg (sparse attention aggregation).

Math (see reference):
  Q = main @ Wq.T + bq                     [2048, 512]
  K = other @ Wk.T + bk                    [2048, 512]
  attn = softmax(where(mask, -BIG, Q K.T / sqrt(512)), axis=-1)   [2048, 2048]
  out[b, m, k] = sum_o attn[m, o] * fix[b, o] * other[o, k]       [32, 2048, 512]

Sharding: rows of `main` (the m axis) are split 256-per-core across 8 cores —
attention and the big einsum shard perfectly with zero collectives; only the
K projection (~1 GFLOP) is replicated.

The dominant aggregation (17.2 GFLOP/core) runs transposed —
  out.T[k, (j, m)] += other[o, k] * wf_b[o, m],   wf_b = pt * fix[b]
with `other` (batch-invariant) STATIONARY and a batch-PAIR of unnormalized
wf as the 512-wide moving operand; softmax 1/rowsum is applied at the PSUM
drain via a row-broadcast reciprocal.  Five of the eight o-tile PAIRS run
in fp8-e4m3 DoubleRow (2 o-rows per PE cell, ~1.8x over f32r), the rest in
f32r; the pair set {0,1,3,5,6} was chosen by per-pair error simulation —
rel-err 1.85e-2 simulated / 1.874e-2 measured, under the 2e-2 gate (any
6-pair set exceeds it).  wf8 production is split ACT/DVE (DVE fp8-out
tensor_scalar measures ~300ns, same as f32-out — unlike its 3-6x-slow
bf16-out path).  The softmax denominator uses reciprocal_approx_fast on a
PE-broadcast rowsum row (DVE's exact reciprocal is ~8 cyc/elem and stalled
the agg start).  Measured 222.6us vs the 307.8us baseline.
Engine assignment follows measured per-op costs: fp8 wf tiles on ACT (~0.6us,
native dtype convert), f32 wf tiles on DVE (~0.28us — any 1/2-byte DVE
output runs 3-6x slower, so DVE stays in f32), drains on DVE as
tensor_tensor with the recip matrix, gpsimd only for partition_broadcast.
"""

import math
import os
import sys

import numpy as np
import ml_dtypes

if "/opt/trn_rl_repo" not in sys.path:
    sys.path.insert(0, "/opt/trn_rl_repo")

import concourse.bass as bass
import concourse.tile as tile
from concourse import bacc, mybir
from concourse.bass_utils import run_bass_kernel_spmd

F32 = mybir.dt.float32
F32R = mybir.dt.float32r
FP8 = mybir.dt.float8e4
U8 = mybir.dt.uint8
AF = mybir.ActivationFunctionType
MUL = mybir.AluOpType.mult
DR = mybir.MatmulPerfMode.DoubleRow

N_CORES = 8
M, O, D = 2048, 2048, 512       # main rows, other rows, qdim=kdim=mid
B = 32                          # batch
MC = M // N_CORES               # 256 main rows per core
P = 128
NDT = D // P                    # 4 tiles along the 512 dims
NOT = O // P                    # 16 tiles along o
# fp8 o-tile PAIRS (DoubleRow packs 2 o-tiles/matmul); this 5-pair set has
# numpy-simulated rel-err 1.85e-2 vs the 2e-2 gate (6 pairs: 2.04e-2 fails)
F8_PAIRS = (0, 1, 3, 5, 6)
NTP = len(F8_PAIRS)
F8_OTS = tuple(2 * t + s for t in F8_PAIRS for s in range(2))
FR_OTS = tuple(ot for ot in range(NOT) if ot not in F8_OTS)
NFR = len(FR_OTS)               # f32r o-tiles
KC = D // P                     # 4 output k blocks
NBP = B // 2                    # 16 batch pairs
BG = 4                          # batch pairs per psum group
N_WARM = 40                     # dummy matmuls to warm the PE clock gate

_CACHE = {}
LAST_RESULTS = None             # test harness reads exec_time_ns from here


def _build():
    nc = bacc.Bacc("TRN2", target_bir_lowering=False, debug=False,
                   num_devices=N_CORES)

    d_mainT = nc.dram_tensor("mainT", [P, NDT * MC], F32R,
                             kind="ExternalInput").ap()
    d_wqT = nc.dram_tensor("wqT", [P, NDT * D], F32R,
                           kind="ExternalInput").ap()
    d_bq = nc.dram_tensor("bq", [P, NDT], F32, kind="ExternalInput").ap()
    d_wkT = nc.dram_tensor("wkT", [P, NDT * D], F32R,
                           kind="ExternalInput").ap()
    d_bk = nc.dram_tensor("bk", [P, NDT], F32, kind="ExternalInput").ap()
    d_otherT = nc.dram_tensor("otherT", [P, NDT * O], F32R,
                              kind="ExternalInput").ap()   # fc-major
    d_otherS = nc.dram_tensor("otherS", [P, NFR * D], F32R,
                              kind="ExternalInput").ap()   # o-tiles 8..15
    d_o8 = nc.dram_tensor("o8", [P, NTP, 2, D], FP8,
                          kind="ExternalInput").ap()       # o-tiles 0..7
    d_fixT = nc.dram_tensor("fixT", [P, NOT * B], F32,
                            kind="ExternalInput").ap()
    d_maskT = nc.dram_tensor("maskT", [P, NOT * MC], U8,
                             kind="ExternalInput").ap()
    d_out = nc.dram_tensor("out", [NBP, P, KC, 2, MC], F32,
                           kind="ExternalOutput").ap()

    with tile.TileContext(nc) as tc:
        with tc.tile_pool(name="persist", bufs=1) as pp:
            with tc.tile_pool(name="qk", bufs=1) as qk:
                # ---- loads, in dependency order -----------------------
                with tc.tile_pool(name="proj", bufs=1) as proj, \
                     tc.tile_pool(name="psqk", bufs=3, space="PSUM") as psqk:
                    # DMA order = PE dependency order: KT (the long phase)
                    # runs first, so wk+otP stream first; QT's wq/mt stream
                    # during KT; agg-only tensors (otherS/o8/fix) last.
                    wkP = proj.tile([P, NDT * D], F32R, name="wkP", tag="wkP")
                    nc.sync.dma_start(wkP[:, 0:P], d_wkT[:, 0:P])  # warm gate
                    nc.sync.dma_start(wkP[:, P:D], d_wkT[:, P:D])
                    nc.sync.dma_start(wkP[:, D:2 * D], d_wkT[:, D:2 * D])
                    nc.sync.dma_start(wkP[:, 2 * D:NDT * D],
                                      d_wkT[:, 2 * D:NDT * D])
                    otP = proj.tile([P, NDT * O], F32R, name="otP", tag="otP")
                    nc.sync.dma_start(otP[:, 0:O // 2], d_otherT[:, 0:O // 2])
                    nc.sync.dma_start(otP[:, O // 2:O],
                                      d_otherT[:, O // 2:O])
                    bkP = proj.tile([P, NDT], F32, name="bkP", tag="bkP")
                    nc.sync.dma_start(bkP[:], d_bk[:])
                    wqP = proj.tile([P, NDT * D], F32R, name="wqP", tag="wqP")
                    nc.sync.dma_start(wqP[:, 0:2 * D], d_wqT[:, 0:2 * D])
                    nc.sync.dma_start(wqP[:, 2 * D:NDT * D],
                                      d_wqT[:, 2 * D:NDT * D])
                    mtP = proj.tile([P, NDT * MC], F32R, name="mtP", tag="mtP")
                    nc.sync.dma_start(mtP[:], d_mainT[:])
                    bqP = proj.tile([P, NDT], F32, name="bqP", tag="bqP")
                    nc.sync.dma_start(bqP[:], d_bq[:])
                    maskP = pp.tile([P, NOT * MC], U8, name="maskP",
                                    tag="maskP")
                    nc.sync.dma_start(maskP[:], d_maskT[:])
                    for fc in range(1, NDT):  # fc-major chunks pipeline w/ KT
                        nc.sync.dma_start(otP[:, fc * O:(fc + 1) * O],
                                          d_otherT[:, fc * O:(fc + 1) * O])
                    otherS = pp.tile([P, NFR * D], F32R, name="otherS",
                                     tag="otherS")
                    for q in range(NFR // 2):
                        nc.sync.dma_start(
                            otherS[:, q * 2 * D:(q + 1) * 2 * D],
                            d_otherS[:, q * 2 * D:(q + 1) * 2 * D])
                    o8sb = pp.tile([P, NTP, 2, D], FP8, name="o8sb",
                                   tag="o8sb")
                    nc.sync.dma_start(o8sb[:], d_o8[:])
                    fixP = pp.tile([P, NOT * B], F32, name="fixP", tag="fixP")
                    nc.sync.dma_start(fixP[:], d_fixT[:])

                    qt_sb = [qk.tile([P, MC], F32, name=f"qt{i}", tag=f"qt{i}")
                             for i in range(NDT)]
                    kt_sb = [qk.tile([P, O], F32, name=f"kt{i}", tag=f"kt{i}")
                             for i in range(NDT)]
                    # pt stored as adjacent-ot PAIR tiles [P, 2, MC]: the
                    # attn softmax runs on [128,512] pair tiles (half the
                    # STT/exp/rowsum ops); slice [:, ot%2, :] to read one ot
                    ptp_sb = [pp.tile([P, 2, MC], F32, name=f"pt{i}",
                                      tag=f"pt{i}") for i in range(NOT // 2)]

                    def pt_at(ot):
                        return ptp_sb[ot // 2][:, ot % 2, :]
                    ones_sb = pp.tile([P, P], F32, name="ones", tag="ones")
                    nc.vector.memset(ones_sb[:], 1.0)
                    rrow = pp.tile([P, MC], F32, name="rrow", tag="rrow")
                    nc.vector.memset(rrow[:], 0.0)
                    recipM2 = pp.tile([P, 2, MC], F32, name="recipM2",
                                      tag="recipM2")

                    # ---- PE warmup ------------------------------------
                    warm_ps = psqk.tile([P, P], F32, name="warm_ps",
                                        tag="warm", bufs=1)
                    for _ in range(N_WARM):
                        nc.tensor.matmul(warm_ps[:], wkP[:, 0:P], wkP[:, 0:P],
                                         start=True, stop=True)

                    # ---- KT[mid, o] = wkT.T @ otherT + bk -------------
                    for fc in range(NDT):
                        for pt in range(NDT):
                            ps = psqk.tile([P, D], F32, name="psk", tag="psk")
                            for ct in range(NDT):
                                nc.tensor.matmul(
                                    ps[:],
                                    wkP[:, ct * D + pt * P:
                                        ct * D + (pt + 1) * P],
                                    otP[:, fc * O + ct * D:
                                        fc * O + (ct + 1) * D],
                                    start=(ct == 0), stop=(ct == NDT - 1))
                            nc.vector.tensor_scalar_add(
                                kt_sb[pt][:, fc * D:(fc + 1) * D]
                                .bitcast(F32R),
                                ps[:], bkP[:, pt:pt + 1])

                    # ---- QT[mid, m] = wqT.T @ mainT + bq --------------
                    for pt in range(NDT):
                        ps = psqk.tile([P, MC], F32, name="psq", tag="psq")
                        for ct in range(NDT):
                            nc.tensor.matmul(
                                ps[:],
                                wqP[:, ct * D + pt * P:ct * D + (pt + 1) * P],
                                mtP[:, ct * MC:(ct + 1) * MC],
                                start=(ct == 0), stop=(ct == NDT - 1))
                        # ACT is idle here; DVE still has the kt-bias tail,
                        # and the last qt bias gates the attn start
                        nc.scalar.activation(qt_sb[pt][:].bitcast(F32R),
                                             ps[:], AF.Identity,
                                             bias=bqP[:, pt:pt + 1])

                # ---- attnT, exp, rowsum, recip row --------------------
                with tc.tile_pool(name="ps4", bufs=6, space="PSUM") as ps4, \
                     tc.tile_pool(name="psr", bufs=1, space="PSUM") as psr:
                    # rowsum (ones.T @ pt -> [1, MC]) matmuls interleave
                    # with the attn chains to fill PE gaps
                    # f32r-set ot pairs first: their pt tiles gate the
                    # DVE-produced wff of the first agg group
                    rsps = psr.tile([1, MC], F32, name="rsps", tag="rsps")
                    pair_order = ([(o, o + 1) for o in FR_OTS[::2]]
                                  + [(o, o + 1) for o in F8_OTS[::2]])
                    for oa, ob in pair_order:
                        ps = ps4.tile([P, 2, MC], F32, name="psa", tag="psa")
                        for h, ot in enumerate((oa, ob)):
                            for ct in range(NDT):
                                nc.tensor.matmul(
                                    ps[:, h, :],
                                    kt_sb[ct][:, ot * P:(ot + 1) * P]
                                    .bitcast(F32R),
                                    qt_sb[ct][:].bitcast(F32R),
                                    start=(ct == 0), stop=(ct == NDT - 1))
                        # psa += mask * -1e9; exp underflows masked to 0
                        nc.vector.scalar_tensor_tensor(
                            ps[:, :, :],
                            maskP[:, oa * MC:(oa + 2) * MC], -1.0e9,
                            ps[:, :, :], op0=MUL, op1=mybir.AluOpType.add)
                        nc.scalar.activation(
                            ptp_sb[oa // 2][:].bitcast(F32R), ps[:, :, :],
                            AF.Exp)
                    for i, (oa, ob) in enumerate(pair_order):
                        for h in range(2):
                            nc.tensor.matmul(
                                rsps[:], ones_sb[:, 0:1].bitcast(F32R),
                                ptp_sb[oa // 2][:, h, :].bitcast(F32R),
                                start=(i == 0 and h == 0),
                                stop=(i == NOT // 2 - 1 and h == 1))
                    # rrow is zero except row 0 = rowsum, so ones.T @ rrow
                    # broadcasts the rowsum row to all 128 partitions; the
                    # reciprocal then runs 128-way parallel (a [1,256]
                    # reciprocal is one serial lane, 1.7us)
                    with nc.allow_low_precision(reason="f32r is fp32-width"):
                        nc.vector.tensor_copy(rrow[0:1, :].bitcast(F32R),
                                              rsps[:])
                    psB = psr.tile([P, MC], F32, name="psB", tag="psB")
                    nc.tensor.matmul(psB[:], ones_sb[:].bitcast(F32R),
                                     rrow[:].bitcast(F32R),
                                     start=True, stop=True)
                    nc.vector.reciprocal_approx_fast(recipM2[:, 0, :],
                                                     psB[:])
                    nc.vector.tensor_copy(recipM2[:, 1, :],
                                          recipM2[:, 0, :])

            # ---- weighted aggregation ---------------------------------
            # out.T[k, (j, m)] per batch-pair: stationary = other tiles,
            # moving = unnormalized wf = pt * fix[b]; fp8 DoubleRow for
            # o-tiles 0..7 (ACT-generated wf8), f32r for 8..15 (DVE wf).
            with tc.tile_pool(name="wf8p", bufs=8) as wf8p, \
                 tc.tile_pool(name="wffp", bufs=6) as wffp, \
                 tc.tile_pool(name="outp", bufs=8) as outp, \
                 tc.tile_pool(name="aggps", bufs=8, space="PSUM") as aggps:
                groups = []
                nxt = 0
                for sz in (2, 2, 3, 3, 3, 3):  # ramp: groups small enough
                    groups.append(range(nxt, nxt + sz))   # that the wf
                    nxt += sz                             # trickle keeps up
                for bps in groups:
                    wf8_t, wff_t = {}, {}
                    for bp in bps:
                        wf8_t[bp] = wf8p.tile([P, NTP, 2, 2, MC], FP8,
                                              name="wf8", tag="wf8")
                        wff_t[bp] = wffp.tile([P, NFR, 2, MC], F32R,
                                              name="wff", tag="wff")
                    # f32 wf on DVE (consumed first by the matmul chains)
                    for of in range(NFR):
                        ot = FR_OTS[of]
                        for bp in bps:
                            for j in range(2):
                                b = 2 * bp + j
                                nc.vector.tensor_scalar_mul(
                                    wff_t[bp][:, of, j, :], pt_at(ot),
                                    fixP[:, ot * B + b:ot * B + b + 1])
                    # wf8 on ACT (fp8 out), in MM consumption order; the
                    # last pair's (s=1, j=1) op goes to DVE — ACT is ~96%
                    # busy in the agg window and DVE has slack
                    for tp in range(NTP):
                        for bp in bps:
                            for s in range(2):
                                ot = 2 * F8_PAIRS[tp] + s
                                for j in range(2):
                                    b = 2 * bp + j
                                    dst = wf8_t[bp][:, tp, s, j, :]
                                    fx = fixP[:, ot * B + b:ot * B + b + 1]
                                    if s == 1:
                                        nc.vector.tensor_scalar_mul(
                                            dst, pt_at(ot), fx)
                                    else:
                                        nc.scalar.activation(
                                            dst, pt_at(ot), AF.Copy,
                                            scale=fx)
                    for kc in range(KC):
                        ps_g = {}
                        for bp in bps:
                            ps_g[bp] = aggps.tile([P, 2 * MC], F32,
                                                  name="pso", tag="pso")
                        for of in range(NFR):
                            lhs = otherS[:, of * D + kc * P:
                                         of * D + (kc + 1) * P]
                            for bp in bps:
                                nc.tensor.matmul(
                                    ps_g[bp][:], lhs,
                                    wff_t[bp][:, of, :, :],
                                    start=(of == 0), stop=False)
                        for tp in range(NTP):
                            lhs = o8sb[:, tp, :, kc * P:(kc + 1) * P]
                            for bp in bps:
                                nc.tensor.matmul(
                                    ps_g[bp][:], lhs,
                                    wf8_t[bp][:, tp, :, :, :],
                                    start=False, stop=(tp == NTP - 1),
                                    perf_mode=DR)
                        for bp in bps:
                            ob = outp.tile([P, 2, MC], F32, name="osb",
                                           tag="osb")
                            nc.vector.tensor_tensor(
                                ob[:], ps_g[bp][:], recipM2[:], op=MUL)
                            nc.sync.dma_start(d_out[bp][:, kc, :, :], ob[:])

    nc.compile()
    return nc


def _pack(a, ntiles, width):
    """[ntiles*128, width] -> [128, ntiles*width] partition-packed layout."""
    return np.ascontiguousarray(
        a.reshape(ntiles, P, width).transpose(1, 0, 2).reshape(P, -1))


def kernel(main_feat, other_feat, fix_feat, mask, Wq, bq, Wk, bk):
    global LAST_RESULTS
    main_feat = np.asarray(main_feat, dtype=np.float32)
    other_feat = np.asarray(other_feat, dtype=np.float32)
    fix_feat = np.asarray(fix_feat, dtype=np.float32)
    mask = np.asarray(mask)
    Wq = np.asarray(Wq, dtype=np.float32)
    bq = np.asarray(bq, dtype=np.float32)
    Wk = np.asarray(Wk, dtype=np.float32)
    bk = np.asarray(bk, dtype=np.float32)

    if "nc" not in _CACHE:
        _CACHE["nc"] = _build()
    nc = _CACHE["nc"]

    inv = np.float32(1.0 / math.sqrt(D))
    wqT = _pack(Wq.T * inv, NDT, D)                   # scale folded into Wq
    bq_p = _pack((bq * inv).reshape(D, 1), NDT, 1)
    wkT = _pack(np.ascontiguousarray(Wk.T), NDT, D)
    bk_p = _pack(bk.reshape(D, 1), NDT, 1)
    # otherT fc-major: [p, fc*O + ct*D + oo] = other.T[ct*128+p, fc*D+oo]
    otherT = np.ascontiguousarray(
        other_feat.T.reshape(NDT, P, NDT, D).transpose(1, 2, 0, 3)
        .reshape(P, NDT * O))
    # f32r o-tiles packed in FR_OTS order; fp8 pair-tiles in F8_PAIRS order
    otherS = _pack(np.concatenate(
        [other_feat[ot * P:(ot + 1) * P] for ot in FR_OTS]), NFR, D)
    o8 = np.ascontiguousarray(
        np.stack([other_feat[2 * t * P:(2 * t + 2) * P].astype(
            ml_dtypes.float8_e4m3).reshape(2, P, D) for t in F8_PAIRS])
        .transpose(2, 0, 1, 3))
    fixT = _pack(np.ascontiguousarray(fix_feat.T), NOT, B)
    mainT = main_feat.T                               # [D, M] view
    mask_u8 = mask.astype(np.uint8)                   # [M, O]

    in_maps = []
    for c in range(N_CORES):
        sl = slice(c * MC, (c + 1) * MC)
        in_maps.append({
            "mainT": _pack(np.ascontiguousarray(mainT[:, sl]), NDT, MC),
            "wqT": wqT, "bq": bq_p, "wkT": wkT, "bk": bk_p,
            "otherT": otherT, "otherS": otherS, "o8": o8,
            "fixT": fixT,
            "maskT": _pack(np.ascontiguousarray(mask_u8[sl, :].T), NOT, MC),
        })

    try:
        res = run_bass_kernel_spmd(nc, in_maps, core_ids=list(range(N_CORES)))
    except Exception:
        # The BASS_TRACE=1 profiling path needs antenv.axon_hooks + artifact
        # upload, which not every image carries — rerun without tracing.
        if os.environ.get("BASS_NEVER_TRACE") == "1":
            raise
        os.environ["BASS_NEVER_TRACE"] = "1"
        res = run_bass_kernel_spmd(nc, in_maps, core_ids=list(range(N_CORES)))
    LAST_RESULTS = res
    # device layout is [NBP, P, KC, 2, MC] per core: out.T blocks
    # out[2bp+j, mc_base+m, kc*128+p] = dev[bp, p, kc, j, m]
    parts = []
    for c in range(N_CORES):
        arr = res.results[c]["out"]                  # [16, 128, 4, 2, 256]
        parts.append(np.ascontiguousarray(
            arr.transpose(0, 3, 4, 2, 1)).reshape(B, MC, D))
    return np.concatenate(parts, axis=1)


# revision 59
# speedup vs baseline: 1.0006x; 1.0006x over previous
"""Trainium2 8-core kernel for nn_AttnAgg (sparse attention aggregation).

Math (see reference):
  Q = main @ Wq.T + bq                     [2048, 512]
  K = other @ Wk.T + bk                    [2048, 512]
  attn = softmax(where(mask, -BIG, Q K.T / sqrt(512)), axis=-1)   [2048, 2048]
  out[b, m, k] = sum_o attn[m, o] * fix[b, o] * other[o, k]       [32, 2048, 512]

Sharding: rows of `main` (the m axis) are split 256-per-core across 8 cores —
attention and the big einsum shard perfectly with zero collectives; only the
K projection (~1 GFLOP) is replicated.

The dominant aggregation (17.2 GFLOP/core) runs transposed —
  out.T[k, (j, m)] += other[o, k] * wf_b[o, m],   wf_b = pt * fix[b]
with `other` (batch-invariant) STATIONARY and a batch-PAIR of unnormalized
wf as the 512-wide moving operand; softmax 1/rowsum is applied at the PSUM
drain via a row-broadcast reciprocal.  Five of the eight o-tile PAIRS run
in fp8-e4m3 DoubleRow (2 o-rows per PE cell, ~1.8x over f32r), the rest in
f32r; the pair set {0,1,3,5,6} was chosen by per-pair error simulation —
rel-err 1.85e-2 simulated / 1.874e-2 measured, under the 2e-2 gate (any
6-pair set exceeds it).  wf8 production is split ACT/DVE (DVE fp8-out
tensor_scalar measures ~300ns, same as f32-out — unlike its 3-6x-slow
bf16-out path).  The softmax denominator uses reciprocal_approx_fast on a
PE-broadcast rowsum row (DVE's exact reciprocal is ~8 cyc/elem and stalled
the agg start).  Measured 222.6us vs the 307.8us baseline.
Engine assignment follows measured per-op costs: fp8 wf tiles on ACT (~0.6us,
native dtype convert), f32 wf tiles on DVE (~0.28us — any 1/2-byte DVE
output runs 3-6x slower, so DVE stays in f32), drains on DVE as
tensor_tensor with the recip matrix, gpsimd only for partition_broadcast.
"""

import math
import os
import sys

import numpy as np
import ml_dtypes

if "/opt/trn_rl_repo" not in sys.path:
    sys.path.insert(0, "/opt/trn_rl_repo")

import concourse.bass as bass
import concourse.tile as tile
from concourse import bacc, mybir
from concourse.bass_utils import run_bass_kernel_spmd

F32 = mybir.dt.float32
F32R = mybir.dt.float32r
FP8 = mybir.dt.float8e4
U8 = mybir.dt.uint8
AF = mybir.ActivationFunctionType
MUL = mybir.AluOpType.mult
DR = mybir.MatmulPerfMode.DoubleRow

N_CORES = 8
M, O, D = 2048, 2048, 512       # main rows, other rows, qdim=kdim=mid
B = 32                          # batch
MC = M // N_CORES               # 256 main rows per core
P = 128
NDT = D // P                    # 4 tiles along the 512 dims
NOT = O // P                    # 16 tiles along o
# fp8 o-tile PAIRS (DoubleRow packs 2 o-tiles/matmul); this 5-pair set has
# numpy-simulated rel-err 1.85e-2 vs the 2e-2 gate (6 pairs: 2.04e-2 fails)
F8_PAIRS = (0, 1, 3, 5, 6)
NTP = len(F8_PAIRS)
F8_OTS = tuple(2 * t + s for t in F8_PAIRS for s in range(2))
FR_OTS = tuple(ot for ot in range(NOT) if ot not in F8_OTS)
NFR = len(FR_OTS)               # f32r o-tiles
KC = D // P                     # 4 output k blocks
NBP = B // 2                    # 16 batch pairs
BG = 4                          # batch pairs per psum group
N_WARM = 40                     # dummy matmuls to warm the PE clock gate

_CACHE = {}
LAST_RESULTS = None             # test harness reads exec_time_ns from here


def _build():
    nc = bacc.Bacc("TRN2", target_bir_lowering=False, debug=False,
                   num_devices=N_CORES)

    d_mainT = nc.dram_tensor("mainT", [P, NDT * MC], F32R,
                             kind="ExternalInput").ap()
    d_wqT = nc.dram_tensor("wqT", [P, NDT * D], F32R,
                           kind="ExternalInput").ap()
    d_bq = nc.dram_tensor("bq", [P, NDT], F32, kind="ExternalInput").ap()
    d_wkT = nc.dram_tensor("wkT", [P, NDT * D], F32R,
                           kind="ExternalInput").ap()
    d_bk = nc.dram_tensor("bk", [P, NDT], F32, kind="ExternalInput").ap()
    d_otherT = nc.dram_tensor("otherT", [P, NDT * O], F32R,
                              kind="ExternalInput").ap()   # fc-major
    d_otherS = nc.dram_tensor("otherS", [P, NFR * D], F32R,
                              kind="ExternalInput").ap()   # o-tiles 8..15
    d_o8 = nc.dram_tensor("o8", [P, NTP, 2, D], FP8,
                          kind="ExternalInput").ap()       # o-tiles 0..7
    d_fixT = nc.dram_tensor("fixT", [P, NOT * B], F32,
                            kind="ExternalInput").ap()
    d_maskT = nc.dram_tensor("maskT", [P, NOT * MC], U8,
                             kind="ExternalInput").ap()
    d_out = nc.dram_tensor("out", [NBP, P, KC, 2, MC], F32,
                           kind="ExternalOutput").ap()

    with tile.TileContext(nc) as tc:
        with tc.tile_pool(name="persist", bufs=1) as pp:
            with tc.tile_pool(name="qk", bufs=1) as qk:
                # ---- loads, in dependency order -----------------------
                with tc.tile_pool(name="proj", bufs=1) as proj, \
                     tc.tile_pool(name="psqk", bufs=3, space="PSUM") as psqk:
                    # DMA order = PE dependency order: KT (the long phase)
                    # runs first, so wk+otP stream first; QT's wq/mt stream
                    # during KT; agg-only tensors (otherS/o8/fix) last.
                    wkP = proj.tile([P, NDT * D], F32R, name="wkP", tag="wkP")
                    nc.sync.dma_start(wkP[:, 0:P], d_wkT[:, 0:P])  # warm gate
                    nc.sync.dma_start(wkP[:, P:D], d_wkT[:, P:D])
                    nc.sync.dma_start(wkP[:, D:2 * D], d_wkT[:, D:2 * D])
                    nc.sync.dma_start(wkP[:, 2 * D:NDT * D],
                                      d_wkT[:, 2 * D:NDT * D])
                    otP = proj.tile([P, NDT * O], F32R, name="otP", tag="otP")
                    nc.sync.dma_start(otP[:, 0:O // 2], d_otherT[:, 0:O // 2])
                    nc.sync.dma_start(otP[:, O // 2:O],
                                      d_otherT[:, O // 2:O])
                    nc.sync.dma_start(otP[:, O:2 * O], d_otherT[:, O:2 * O])
                    bkP = proj.tile([P, NDT], F32, name="bkP", tag="bkP")
                    nc.sync.dma_start(bkP[:], d_bk[:])
                    wqP = proj.tile([P, NDT * D], F32R, name="wqP", tag="wqP")
                    nc.sync.dma_start(wqP[:, 0:2 * D], d_wqT[:, 0:2 * D])
                    nc.sync.dma_start(wqP[:, 2 * D:NDT * D],
                                      d_wqT[:, 2 * D:NDT * D])
                    mtP = proj.tile([P, NDT * MC], F32R, name="mtP", tag="mtP")
                    nc.sync.dma_start(mtP[:], d_mainT[:])
                    bqP = proj.tile([P, NDT], F32, name="bqP", tag="bqP")
                    nc.sync.dma_start(bqP[:], d_bq[:])
                    for fc in range(2, NDT):  # fc-major chunks pipeline w/ KT
                        nc.sync.dma_start(otP[:, fc * O:(fc + 1) * O],
                                          d_otherT[:, fc * O:(fc + 1) * O])
                    # mask is first needed by the attn STT, well after KT
                    maskP = pp.tile([P, NOT * MC], U8, name="maskP",
                                    tag="maskP")
                    nc.sync.dma_start(maskP[:], d_maskT[:])
                    otherS = pp.tile([P, NFR * D], F32R, name="otherS",
                                     tag="otherS")
                    for q in range(NFR // 2):
                        nc.sync.dma_start(
                            otherS[:, q * 2 * D:(q + 1) * 2 * D],
                            d_otherS[:, q * 2 * D:(q + 1) * 2 * D])
                    o8sb = pp.tile([P, NTP, 2, D], FP8, name="o8sb",
                                   tag="o8sb")
                    nc.sync.dma_start(o8sb[:], d_o8[:])
                    fixP = pp.tile([P, NOT * B], F32, name="fixP", tag="fixP")
                    nc.sync.dma_start(fixP[:], d_fixT[:])

                    qt_sb = [qk.tile([P, MC], F32, name=f"qt{i}", tag=f"qt{i}")
                             for i in range(NDT)]
                    kt_sb = [qk.tile([P, O], F32, name=f"kt{i}", tag=f"kt{i}")
                             for i in range(NDT)]
                    # pt stored as adjacent-ot PAIR tiles [P, 2, MC]: the
                    # attn softmax runs on [128,512] pair tiles (half the
                    # STT/exp/rowsum ops); slice [:, ot%2, :] to read one ot
                    ptp_sb = [pp.tile([P, 2, MC], F32, name=f"pt{i}",
                                      tag=f"pt{i}") for i in range(NOT // 2)]

                    def pt_at(ot):
                        return ptp_sb[ot // 2][:, ot % 2, :]
                    ones_sb = pp.tile([P, P], F32, name="ones", tag="ones")
                    nc.vector.memset(ones_sb[:], 1.0)
                    rrow = pp.tile([P, MC], F32, name="rrow", tag="rrow")
                    nc.vector.memset(rrow[:], 0.0)
                    recipM2 = pp.tile([P, 2, MC], F32, name="recipM2",
                                      tag="recipM2")

                    # ---- PE warmup ------------------------------------
                    warm_ps = psqk.tile([P, P], F32, name="warm_ps",
                                        tag="warm", bufs=1)
                    for _ in range(N_WARM):
                        nc.tensor.matmul(warm_ps[:], wkP[:, 0:P], wkP[:, 0:P],
                                         start=True, stop=True)

                    # ---- KT[mid, o] = wkT.T @ otherT + bk -------------
                    for fc in range(NDT):
                        for pt in range(NDT):
                            ps = psqk.tile([P, D], F32, name="psk", tag="psk")
                            for ct in range(NDT):
                                nc.tensor.matmul(
                                    ps[:],
                                    wkP[:, ct * D + pt * P:
                                        ct * D + (pt + 1) * P],
                                    otP[:, fc * O + ct * D:
                                        fc * O + (ct + 1) * D],
                                    start=(ct == 0), stop=(ct == NDT - 1))
                            nc.vector.tensor_scalar_add(
                                kt_sb[pt][:, fc * D:(fc + 1) * D]
                                .bitcast(F32R),
                                ps[:], bkP[:, pt:pt + 1])

                    # ---- QT[mid, m] = wqT.T @ mainT + bq --------------
                    for pt in range(NDT):
                        ps = psqk.tile([P, MC], F32, name="psq", tag="psq")
                        for ct in range(NDT):
                            nc.tensor.matmul(
                                ps[:],
                                wqP[:, ct * D + pt * P:ct * D + (pt + 1) * P],
                                mtP[:, ct * MC:(ct + 1) * MC],
                                start=(ct == 0), stop=(ct == NDT - 1))
                        # ACT is idle here; DVE still has the kt-bias tail,
                        # and the last qt bias gates the attn start
                        nc.scalar.activation(qt_sb[pt][:].bitcast(F32R),
                                             ps[:], AF.Identity,
                                             bias=bqP[:, pt:pt + 1])

                # ---- attnT, exp, rowsum, recip row --------------------
                with tc.tile_pool(name="ps4", bufs=6, space="PSUM") as ps4, \
                     tc.tile_pool(name="psr", bufs=1, space="PSUM") as psr:
                    # rowsum (ones.T @ pt -> [1, MC]) matmuls interleave
                    # with the attn chains to fill PE gaps
                    # f32r-set ot pairs first: their pt tiles gate the
                    # DVE-produced wff of the first agg group
                    rsps = psr.tile([1, MC], F32, name="rsps", tag="rsps")
                    pair_order = ([(o, o + 1) for o in FR_OTS[::2]]
                                  + [(o, o + 1) for o in F8_OTS[::2]])
                    for oa, ob in pair_order:
                        ps = ps4.tile([P, 2, MC], F32, name="psa", tag="psa")
                        for h, ot in enumerate((oa, ob)):
                            for ct in range(NDT):
                                nc.tensor.matmul(
                                    ps[:, h, :],
                                    kt_sb[ct][:, ot * P:(ot + 1) * P]
                                    .bitcast(F32R),
                                    qt_sb[ct][:].bitcast(F32R),
                                    start=(ct == 0), stop=(ct == NDT - 1))
                        # psa += mask * -1e9; exp underflows masked to 0
                        nc.vector.scalar_tensor_tensor(
                            ps[:, :, :],
                            maskP[:, oa * MC:(oa + 2) * MC], -1.0e9,
                            ps[:, :, :], op0=MUL, op1=mybir.AluOpType.add)
                        nc.scalar.activation(
                            ptp_sb[oa // 2][:].bitcast(F32R), ps[:, :, :],
                            AF.Exp)
                    for i, (oa, ob) in enumerate(pair_order):
                        for h in range(2):
                            nc.tensor.matmul(
                                rsps[:], ones_sb[:, 0:1].bitcast(F32R),
                                ptp_sb[oa // 2][:, h, :].bitcast(F32R),
                                start=(i == 0 and h == 0),
                                stop=(i == NOT // 2 - 1 and h == 1))
                    # rrow is zero except row 0 = rowsum, so ones.T @ rrow
                    # broadcasts the rowsum row to all 128 partitions; the
                    # reciprocal then runs 128-way parallel (a [1,256]
                    # reciprocal is one serial lane, 1.7us)
                    with nc.allow_low_precision(reason="f32r is fp32-width"):
                        nc.vector.tensor_copy(rrow[0:1, :].bitcast(F32R),
                                              rsps[:])
                    psB = psr.tile([P, MC], F32, name="psB", tag="psB")
                    nc.tensor.matmul(psB[:], ones_sb[:].bitcast(F32R),
                                     rrow[:].bitcast(F32R),
                                     start=True, stop=True)
                    nc.vector.reciprocal_approx_fast(recipM2[:, 0, :],
                                                     psB[:])
                    nc.vector.tensor_copy(recipM2[:, 1, :],
                                          recipM2[:, 0, :])

            # ---- weighted aggregation ---------------------------------
            # out.T[k, (j, m)] per batch-pair: stationary = other tiles,
            # moving = unnormalized wf = pt * fix[b]; fp8 DoubleRow for
            # o-tiles 0..7 (ACT-generated wf8), f32r for 8..15 (DVE wf).
            with tc.tile_pool(name="wf8p", bufs=8) as wf8p, \
                 tc.tile_pool(name="wffp", bufs=6) as wffp, \
                 tc.tile_pool(name="outp", bufs=8) as outp, \
                 tc.tile_pool(name="aggps", bufs=8, space="PSUM") as aggps:
                groups = []
                nxt = 0
                for sz in (2, 2, 3, 3, 3, 3):  # ramp: groups small enough
                    groups.append(range(nxt, nxt + sz))   # that the wf
                    nxt += sz                             # trickle keeps up
                for bps in groups:
                    wf8_t, wff_t = {}, {}
                    for bp in bps:
                        wf8_t[bp] = wf8p.tile([P, NTP, 2, 2, MC], FP8,
                                              name="wf8", tag="wf8")
                        wff_t[bp] = wffp.tile([P, NFR, 2, MC], F32R,
                                              name="wff", tag="wff")
                    # f32 wf on DVE (consumed first by the matmul chains)
                    for of in range(NFR):
                        ot = FR_OTS[of]
                        for bp in bps:
                            for j in range(2):
                                b = 2 * bp + j
                                nc.vector.tensor_scalar_mul(
                                    wff_t[bp][:, of, j, :], pt_at(ot),
                                    fixP[:, ot * B + b:ot * B + b + 1])
                    # wf8 on ACT (fp8 out), in MM consumption order; the
                    # last pair's (s=1, j=1) op goes to DVE — ACT is ~96%
                    # busy in the agg window and DVE has slack
                    for tp in range(NTP):
                        for bp in bps:
                            for s in range(2):
                                ot = 2 * F8_PAIRS[tp] + s
                                for j in range(2):
                                    b = 2 * bp + j
                                    dst = wf8_t[bp][:, tp, s, j, :]
                                    fx = fixP[:, ot * B + b:ot * B + b + 1]
                                    if s == 1:
                                        nc.vector.tensor_scalar_mul(
                                            dst, pt_at(ot), fx)
                                    else:
                                        nc.scalar.activation(
                                            dst, pt_at(ot), AF.Copy,
                                            scale=fx)
                    for kc in range(KC):
                        ps_g = {}
                        for bp in bps:
                            ps_g[bp] = aggps.tile([P, 2 * MC], F32,
                                                  name="pso", tag="pso")
                        for of in range(NFR):
                            lhs = otherS[:, of * D + kc * P:
                                         of * D + (kc + 1) * P]
                            for bp in bps:
                                nc.tensor.matmul(
                                    ps_g[bp][:], lhs,
                                    wff_t[bp][:, of, :, :],
                                    start=(of == 0), stop=False)
                        for tp in range(NTP):
                            lhs = o8sb[:, tp, :, kc * P:(kc + 1) * P]
                            for bp in bps:
                                nc.tensor.matmul(
                                    ps_g[bp][:], lhs,
                                    wf8_t[bp][:, tp, :, :, :],
                                    start=False, stop=(tp == NTP - 1),
                                    perf_mode=DR)
                        for bp in bps:
                            ob = outp.tile([P, 2, MC], F32, name="osb",
                                           tag="osb")
                            nc.vector.tensor_tensor(
                                ob[:], ps_g[bp][:], recipM2[:], op=MUL)
                            nc.sync.dma_start(d_out[bp][:, kc, :, :], ob[:])

    nc.compile()
    return nc


def _pack(a, ntiles, width):
    """[ntiles*128, width] -> [128, ntiles*width] partition-packed layout."""
    return np.ascontiguousarray(
        a.reshape(ntiles, P, width).transpose(1, 0, 2).reshape(P, -1))


def kernel(main_feat, other_feat, fix_feat, mask, Wq, bq, Wk, bk):
    global LAST_RESULTS
    main_feat = np.asarray(main_feat, dtype=np.float32)
    other_feat = np.asarray(other_feat, dtype=np.float32)
    fix_feat = np.asarray(fix_feat, dtype=np.float32)
    mask = np.asarray(mask)
    Wq = np.asarray(Wq, dtype=np.float32)
    bq = np.asarray(bq, dtype=np.float32)
    Wk = np.asarray(Wk, dtype=np.float32)
    bk = np.asarray(bk, dtype=np.float32)

    if "nc" not in _CACHE:
        _CACHE["nc"] = _build()
    nc = _CACHE["nc"]

    inv = np.float32(1.0 / math.sqrt(D))
    wqT = _pack(Wq.T * inv, NDT, D)                   # scale folded into Wq
    bq_p = _pack((bq * inv).reshape(D, 1), NDT, 1)
    wkT = _pack(np.ascontiguousarray(Wk.T), NDT, D)
    bk_p = _pack(bk.reshape(D, 1), NDT, 1)
    # otherT fc-major: [p, fc*O + ct*D + oo] = other.T[ct*128+p, fc*D+oo]
    otherT = np.ascontiguousarray(
        other_feat.T.reshape(NDT, P, NDT, D).transpose(1, 2, 0, 3)
        .reshape(P, NDT * O))
    # f32r o-tiles packed in FR_OTS order; fp8 pair-tiles in F8_PAIRS order
    otherS = _pack(np.concatenate(
        [other_feat[ot * P:(ot + 1) * P] for ot in FR_OTS]), NFR, D)
    o8 = np.ascontiguousarray(
        np.stack([other_feat[2 * t * P:(2 * t + 2) * P].astype(
            ml_dtypes.float8_e4m3).reshape(2, P, D) for t in F8_PAIRS])
        .transpose(2, 0, 1, 3))
    fixT = _pack(np.ascontiguousarray(fix_feat.T), NOT, B)
    mainT = main_feat.T                               # [D, M] view
    mask_u8 = mask.astype(np.uint8)                   # [M, O]

    in_maps = []
    for c in range(N_CORES):
        sl = slice(c * MC, (c + 1) * MC)
        in_maps.append({
            "mainT": _pack(np.ascontiguousarray(mainT[:, sl]), NDT, MC),
            "wqT": wqT, "bq": bq_p, "wkT": wkT, "bk": bk_p,
            "otherT": otherT, "otherS": otherS, "o8": o8,
            "fixT": fixT,
            "maskT": _pack(np.ascontiguousarray(mask_u8[sl, :].T), NOT, MC),
        })

    try:
        res = run_bass_kernel_spmd(nc, in_maps, core_ids=list(range(N_CORES)))
    except Exception:
        # The BASS_TRACE=1 profiling path needs antenv.axon_hooks + artifact
        # upload, which not every image carries — rerun without tracing.
        if os.environ.get("BASS_NEVER_TRACE") == "1":
            raise
        os.environ["BASS_NEVER_TRACE"] = "1"
        res = run_bass_kernel_spmd(nc, in_maps, core_ids=list(range(N_CORES)))
    LAST_RESULTS = res
    # device layout is [NBP, P, KC, 2, MC] per core: out.T blocks
    # out[2bp+j, mc_base+m, kc*128+p] = dev[bp, p, kc, j, m]
    parts = []
    for c in range(N_CORES):
        arr = res.results[c]["out"]                  # [16, 128, 4, 2, 256]
        parts.append(np.ascontiguousarray(
            arr.transpose(0, 3, 4, 2, 1)).reshape(B, MC, D))
    return np.concatenate(parts, axis=1)
